# revision 9
# baseline (speedup 1.0000x reference)
"""Trainium2 Bass kernel for nn_DecoderV2 (LSTM decoder + coverage attention +
50k-vocab logit head), SPMD over 8 NeuronCores, batch-sharded.

Self-contained: host-side numpy does layout/padding/casts only; all FLOPs run
on-device. Returns (logits, attns, coverages, energies) like the reference.
"""
import os
import numpy as np

import concourse.bass as bass
import concourse.mybir as mybir
import concourse.tile as tile
from concourse import bacc
from concourse.masks import make_identity

f32 = mybir.dt.float32
f16 = mybir.dt.float16
AF = mybir.ActivationFunctionType
OP = mybir.AluOpType

B, T, S = 64, 32, 400
E, EH, DH, A, V, P = 300, 512, 512, 512, 50000, 2
NC = 8
BL = B // NC          # 8 batch rows per core
KPAD = 1408           # [emb 0:300 | ones@300 | pad | ctx 384:896 | h 896:1408]
NKT = KPAD // 128     # 11 k-tiles
G4 = 4 * DH           # 2048
VCH = 512             # phase-2 vocab chunk
NVCH = (V + VCH - 1) // VCH   # 98

_cache = {}


def _build_program():
    if "nc" in _cache:
        return _cache["nc"]
    nc = bacc.Bacc("TRN2", target_bir_lowering=False, debug=False)

    # ---------------- DRAM I/O ----------------
    d_embT = nc.dram_tensor("embT", [128, T * 24], f16, kind="ExternalInput")
    d_wgT = nc.dram_tensor("WgT", [128, NKT * G4], f16, kind="ExternalInput")
    d_wrT = nc.dram_tensor("WrT", [128, NKT * DH], f16, kind="ExternalInput")
    d_wqT = nc.dram_tensor("WqT", [128, 4 * A], f16, kind="ExternalInput")
    d_wtrT = nc.dram_tensor("WtrT", [128, 5 * DH], f16, kind="ExternalInput")
    d_ehcT = nc.dram_tensor("ehcT", [128, 5 * 16], f16, kind="ExternalInput")
    d_wpT = nc.dram_tensor("WpT", [128, 16 * 128], f16, kind="ExternalInput")
    d_encT = nc.dram_tensor("encT", [128, 4 * BL * S], f16, kind="ExternalInput")
    d_encS = nc.dram_tensor("encS", [128, BL * 4 * EH], f16, kind="ExternalInput")
    d_wvbd = nc.dram_tensor("wvbd", [128, 32 * 8], f16, kind="ExternalInput")
    d_wcov = nc.dram_tensor("wcovrow", [1, A], f16, kind="ExternalInput")
    d_mask = nc.dram_tensor("maskadd", [BL, S], f32, kind="ExternalInput")
    d_bpre = nc.dram_tensor("bpre", [128, 4], f32, kind="ExternalInput")
    d_wlT = nc.dram_tensor("WlT", [128, 2 * V], f16, kind="ExternalInput")
    d_blg = nc.dram_tensor("blg", [1, V], f16, kind="ExternalInput")

    d_logits = nc.dram_tensor("logits", [T, BL, V], f32, kind="ExternalOutput")
    d_attns = nc.dram_tensor("attns", [T, BL, S], f32, kind="ExternalOutput")
    d_covs = nc.dram_tensor("covs", [T, BL, S], f32, kind="ExternalOutput")
    d_energ = nc.dram_tensor("energies", [T, BL, S], f32, kind="ExternalOutput")

    with tile.TileContext(nc) as tc:
        wp = tc.alloc_tile_pool(name="wp", bufs=1)
        lp = tc.alloc_tile_pool(name="lp", bufs=1)

        embT = lp.tile([128, T * 24], f16); nc.sync.dma_start(embT[:], d_embT.ap())
        wgT = lp.tile([128, NKT * G4], f16); nc.sync.dma_start(wgT[:], d_wgT.ap())
        wrT = lp.tile([128, NKT * DH], f16); nc.sync.dma_start(wrT[:], d_wrT.ap())
        wqT = lp.tile([128, 4 * A], f16); nc.sync.dma_start(wqT[:], d_wqT.ap())
        encS = lp.tile([128, BL * 4 * EH], f16); nc.sync.dma_start(encS[:], d_encS.ap())
        wvbd = lp.tile([128, 32 * 8], f16); nc.sync.dma_start(wvbd[:], d_wvbd.ap())
        wcovrow = lp.tile([1, A], f16); nc.sync.dma_start(wcovrow[:], d_wcov.ap())
        maskadd = lp.tile([BL, S], f32); nc.sync.dma_start(maskadd[:], d_mask.ap())
        bpre = lp.tile([128, 4], f32); nc.sync.dma_start(bpre[:], d_bpre.ap())
        ident = wp.tile([128, 128], f16)
        make_identity(nc, ident[:])
        ones128 = wp.tile([1, 128], f16)
        nc.gpsimd.memset(ones128[:], 1.0)

        pre = lp.tile([128, 4 * BL * S], f16)
        feat = lp.tile([128, 4 * BL * S], f16)

        # persistent state
        hT = wp.tile([128, 32], f16)
        ctxT = wp.tile([128, 32], f16)
        qT16 = wp.tile([128, 32], f16)
        attnT_bd = wp.tile([128, 4 * 72], f16)
        maxoT = wp.tile([128, 2 * 256], f16)
        c8 = wp.tile([BL, DH], f32)
        cov = wp.tile([BL, S], f32)
        covrow = wp.tile([1, BL * S], f16)

        nc.gpsimd.memset(ctxT[:], 0.0)
        nc.gpsimd.memset(attnT_bd[:], 0.0)
        nc.gpsimd.memset(cov[:], 0.0)
        nc.gpsimd.memset(covrow[:], 0.0)

        # ---------------- init: h0/c0 + pre-compute ----------------
        ps = tc.alloc_tile_pool(name="ps", bufs=2, space="PSUM")
        with tc.tile_pool(name="initsb", bufs=1) as isb:
            wtrT = isb.tile([128, 5 * DH], f16); nc.sync.dma_start(wtrT[:], d_wtrT.ap())
            ehcT = isb.tile([128, 5 * 16], f16); nc.sync.dma_start(ehcT[:], d_ehcT.ap())
            wpT = isb.tile([128, 16 * 128], f16); nc.sync.dma_start(wpT[:], d_wpT.ap())
            encT = isb.tile([128, 4 * BL * S], f16); nc.sync.dma_start(encT[:], d_encT.ap())

            # h0T tiles [128, 8] per m:  sum_k WtrT(k,m).T @ ehcT(k, h-cols)
            for m in range(4):
                hp = ps.tile([128, 8], f32, tag="t")
                for k in range(5):
                    kp = 128 if k < 4 else 1
                    nc.tensor.matmul(
                        hp[:], wtrT[0:kp, k * DH + m * 128 : k * DH + m * 128 + 128],
                        ehcT[0:kp, k * 16 : k * 16 + 8],
                        start=(k == 0), stop=(k == 4))
                nc.vector.tensor_copy(hT[:, m * 8 : m * 8 + 8], hp[:])
            # c0 [8, 512]: sum_k ehcT(k, c-cols).T @ WtrT(k-rowtile)
            cp = ps.tile([BL, DH], f32, tag="a")
            for k in range(5):
                kp = 128 if k < 4 else 1
                nc.tensor.matmul(
                    cp[:], ehcT[0:kp, k * 16 + 8 : k * 16 + 16],
                    wtrT[0:kp, k * DH : (k + 1) * DH],
                    start=(k == 0), stop=(k == 4))
            nc.vector.tensor_copy(c8[:], cp[:])

            # pre[a, (b, s)] = W_pre @ enc.T   (k-tiles over EH)
            for m in range(4):
                for b in range(BL):
                    pp = ps.tile([128, S], f32, tag="a")
                    for k in range(4):
                        nc.tensor.matmul(
                            pp[:], wpT[:, (k * 4 + m) * 128 : (k * 4 + m) * 128 + 128],
                            encT[:, k * (BL * S) + b * S : k * (BL * S) + (b + 1) * S],
                            start=(k == 0), stop=(k == 3))
                    nc.scalar.copy(pre[:, m * (BL * S) + b * S : m * (BL * S) + (b + 1) * S], pp[:])

        # ---------------- the T-step recurrence ----------------
        sp = tc.alloc_tile_pool(name="sp", bufs=1)

        def lhsT_for_k(k, t):
            if k < 3:
                return embT[:, t * 24 + k * 8 : t * 24 + k * 8 + 8]
            if k < 7:
                return ctxT[:, (k - 3) * 8 : (k - 3) * 8 + 8]
            return hT[:, (k - 7) * 8 : (k - 7) * 8 + 8]

        for t in range(T):
            # cov -> f16 row (for wcov rank-1); cov==0 at t==0
            cov16 = sp.tile([BL, S], f16, tag="cov16")
            nc.vector.tensor_copy(cov16[:], cov[:])
            nc.sync.dma_start(
                covrow[0:1, :].rearrange("o (b s) -> o b s", b=BL), cov16[:])

            # gates [8, 2048] computed per 512-chunk (i, f, g, o), tanh each
            thg4 = []
            for n in range(4):
                g_ps = ps.tile([BL, 512], f32, tag="a")
                for k in range(NKT):
                    nc.tensor.matmul(
                        g_ps[:], lhsT_for_k(k, t),
                        wgT[:, k * G4 + n * 512 : k * G4 + (n + 1) * 512],
                        start=(k == 0), stop=(k == NKT - 1))
                thn = sp.tile([BL, 512], f16, tag=f"th{n}")
                nc.scalar.activation(thn[:], g_ps[:], AF.Tanh)
                thg4.append(thn)
            thi, thf, thg, tho = thg4

            # LSTM cell (sigmoid via 0.5*tanh(x/2)+0.5; i/f/o pre-scaled by 0.5 in WgT)
            u = sp.tile([BL, DH], f32, tag="u")
            nc.vector.tensor_scalar(u[:], thf[:], 0.5, 0.5, OP.mult, OP.add)
            c1 = sp.tile([BL, DH], f32, tag="c1")
            nc.vector.tensor_mul(c1[:], u[:], c8[:])
            v = sp.tile([BL, DH], f32, tag="v")
            nc.vector.tensor_scalar(v[:], thi[:], 0.5, 0.5, OP.mult, OP.add)
            w = sp.tile([BL, DH], f32, tag="w")
            nc.vector.tensor_mul(w[:], v[:], thg[:])
            nc.vector.tensor_add(c8[:], c1[:], w[:])
            tanc = sp.tile([BL, DH], f16, tag="tanc")
            nc.scalar.activation(tanc[:], c8[:], AF.Tanh)
            so = sp.tile([BL, DH], f32, tag="so")
            nc.vector.tensor_scalar(so[:], tho[:], 0.5, 0.5, OP.mult, OP.add)
            h16 = sp.tile([BL, DH], f16, tag="h16")
            nc.vector.tensor_mul(h16[:], so[:], tanc[:])

            # hT <- h16.T
            for ch in range(4):
                tp = ps.tile([128, 8], f16, tag="t")
                nc.tensor.transpose(tp[:], h16[:, ch * 128 : (ch + 1) * 128], ident[0:8, 0:8])
                nc.vector.tensor_copy(hT[:, ch * 8 : ch * 8 + 8], tp[:])

            # q = h @ Wq.T -> [8, 512] -> qT16 [128a, 8b] (+ b_pre)
            q_ps = ps.tile([BL, A], f32, tag="a")
            for k in range(4):
                nc.tensor.matmul(q_ps[:], hT[:, k * 8 : k * 8 + 8],
                                 wqT[:, k * A : (k + 1) * A],
                                 start=(k == 0), stop=(k == 3))
            q16 = sp.tile([BL, A], f16, tag="q16")
            nc.vector.tensor_copy(q16[:], q_ps[:])
            for m in range(4):
                tq = ps.tile([128, 8], f16, tag="t")
                nc.tensor.transpose(tq[:], q16[:, m * 128 : (m + 1) * 128], ident[0:8, 0:8])
                nc.vector.tensor_scalar(qT16[:, m * 8 : m * 8 + 8], tq[:],
                                        bpre[:, m : m + 1], None, OP.add)

            # feat = tanh(pre + wcov x cov + q x 1)   [a-part, (b, s)]
            for m in range(4):
                for bq in (0, 2, 4, 6):
                    ap = ps.tile([128, 2 * 512], f32, tag="arg")
                    for j in range(2):
                        b = bq + j
                        dst = ap[:, j * 512 : j * 512 + S]
                        nc.tensor.matmul(dst, ident[:],
                                         pre[:, m * (BL * S) + b * S : m * (BL * S) + (b + 1) * S],
                                         start=True, stop=False)
                        nc.tensor.matmul(dst, wcovrow[0:1, m * 128 : (m + 1) * 128],
                                         covrow[0:1, b * S : (b + 1) * S],
                                         start=False, stop=False)
                        nc.tensor.matmul(dst, ident[:],
                                         qT16[:, m * 8 + b : m * 8 + b + 1].broadcast_to((128, S)),
                                         start=False, stop=True)
                    asrc = ap[:].rearrange("p (j n) -> p j n", n=512)[:, :, 0:S]
                    dstf = feat[:, m * (BL * S) + bq * S : m * (BL * S) + (bq + 2) * S]
                    nc.scalar.activation(
                        dstf.rearrange("p (j n) -> p j n", n=S), asrc, AF.Tanh)

            # energy [8, 400] via block-diag wv
            en_ps = ps.tile([BL, S], f32, tag="a")
            idx = 0
            for m in range(4):
                for b in range(BL):
                    nc.tensor.matmul(
                        en_ps[:], wvbd[:, (m * 8 + b) * 8 : (m * 8 + b) * 8 + 8],
                        feat[:, m * (BL * S) + b * S : m * (BL * S) + (b + 1) * S],
                        start=(idx == 0), stop=(idx == 31))
                    idx += 1

            # masked energies -> softmax (no max-subtraction; exp(-1e12)=0)
            e8 = sp.tile([BL, S], f32, tag="e8")
            nc.vector.tensor_add(e8[:], en_ps[:], maskadd[:])
            nc.sync.dma_start(d_energ.ap()[t], e8[:])
            exps = sp.tile([BL, S], f32, tag="exps")
            zsum = sp.tile([BL, 1], f32, tag="zsum")
            nc.scalar.activation(exps[:], e8[:], AF.Exp, accum_out=zsum[:])
            zrec = sp.tile([BL, 1], f32, tag="zrec")
            nc.vector.reciprocal(zrec[:], zsum[:])
            attn = sp.tile([BL, S], f32, tag="attn")
            nc.vector.tensor_scalar(attn[:], exps[:], zrec[:], None, OP.mult)
            nc.sync.dma_start(d_attns.ap()[t], attn[:])

            # coverage output (pre-update) then cov += attn
            nc.sync.dma_start(d_covs.ap()[t], cov[:])
            nc.vector.tensor_add(cov[:], cov[:], attn[:])

            # attn -> f16 -> block-diagonal attnT (col b of slot (ch, b))
            attn16 = sp.tile([BL, S], f16, tag="attn16")
            nc.vector.tensor_copy(attn16[:], attn[:])
            for ch in range(4):
                cw = 128 if ch < 3 else S - 384
                ta = ps.tile([128, 8], f16, tag="t")
                nc.tensor.transpose(ta[0:cw, :], attn16[:, ch * 128 : ch * 128 + cw],
                                    ident[0:8, 0:8])
                dst9 = attnT_bd[0:cw, ch * 72 : ch * 72 + 72].rearrange(
                    "p (b n) -> p b n", n=9)[:, :, 0:1]
                nc.vector.tensor_copy(dst9, ta[0:cw, :].rearrange("p (b o) -> p b o", o=1))

            # context: block-diag attn lhsT -> one [8, EH] psum
            ctx_ps = ps.tile([BL, EH], f32, tag="a")
            cidx = 0
            for b in range(BL):
                for ch in range(4):
                    cw = 128 if ch < 3 else S - 384
                    nc.tensor.matmul(
                        ctx_ps[:],
                        attnT_bd[0:cw, ch * 72 + 8 * b : ch * 72 + 8 * b + 8],
                        encS[0:cw, b * (4 * EH) + ch * EH : b * (4 * EH) + (ch + 1) * EH],
                        start=(cidx == 0), stop=(cidx == 31))
                    cidx += 1
            ctx8 = sp.tile([BL, EH], f16, tag="ctx8")
            nc.vector.tensor_copy(ctx8[:], ctx_ps[:])
            for ch in range(4):
                tcx = ps.tile([128, 8], f16, tag="t")
                nc.tensor.transpose(tcx[:], ctx8[:, ch * 128 : (ch + 1) * 128], ident[0:8, 0:8])
                nc.vector.tensor_copy(ctxT[:, ch * 8 : ch * 8 + 8], tcx[:])

            # readout [8, 512] -> maxout -> maxoT
            r_ps = ps.tile([BL, DH], f32, tag="a")
            for k in range(NKT):
                nc.tensor.matmul(r_ps[:], lhsT_for_k(k, t),
                                 wrT[:, k * DH : (k + 1) * DH],
                                 start=(k == 0), stop=(k == NKT - 1))
            r_sb = sp.tile([BL, DH], f32, tag="rsb")
            nc.vector.tensor_copy(r_sb[:], r_ps[:])
            maxo16 = sp.tile([BL, DH // 2], f16, tag="maxo16")
            nc.vector.tensor_max(maxo16[:], r_sb[:, 0:256], r_sb[:, 256:512])
            for ch in range(2):
                tm = ps.tile([128, 8], f16, tag="t")
                nc.tensor.transpose(tm[:], maxo16[:, ch * 128 : (ch + 1) * 128], ident[0:8, 0:8])
                nc.vector.tensor_copy(maxoT[:, ch * 256 + t * 8 : ch * 256 + t * 8 + 8], tm[:])

        # ---------------- phase 2: logits = maxo @ W_logit.T + b_logit ----------------
        sp.release()
        lp.release()
        lgflat = d_logits.ap().rearrange("t b v -> (t b) v")
        with tc.tile_pool(name="lsb", bufs=3) as lsb:
            for chv in range(NVCH):
                cw = min(VCH, V - chv * VCH)
                w0 = lsb.tile([128, VCH], f16, tag="w0")
                nc.sync.dma_start(w0[:, 0:cw], d_wlT.ap()[:, chv * VCH : chv * VCH + cw])
                w1 = lsb.tile([128, VCH], f16, tag="w1")
                nc.sync.dma_start(w1[:, 0:cw], d_wlT.ap()[:, V + chv * VCH : V + chv * VCH + cw])
                bl = lsb.tile([1, VCH], f16, tag="bl")
                nc.sync.dma_start(bl[0:1, 0:cw], d_blg.ap()[0:1, chv * VCH : chv * VCH + cw])
                for m in range(2):
                    lg_ps = ps.tile([128, VCH], f32, tag="a")
                    nc.tensor.matmul(lg_ps[:, 0:cw], maxoT[:, m * 128 : (m + 1) * 128],
                                     w0[:, 0:cw], start=True, stop=False)
                    nc.tensor.matmul(lg_ps[:, 0:cw], maxoT[:, 256 + m * 128 : 256 + (m + 1) * 128],
                                     w1[:, 0:cw], start=False, stop=False)
                    nc.tensor.matmul(lg_ps[:, 0:cw], ones128[0:1, :], bl[0:1, 0:cw],
                                     start=False, stop=True)
                    lg_sb = lsb.tile([128, VCH], f32, tag="lgsb")
                    if m == 0:
                        nc.vector.tensor_copy(lg_sb[:, 0:cw], lg_ps[:, 0:cw])
                    else:
                        nc.scalar.copy(lg_sb[:, 0:cw], lg_ps[:, 0:cw])
                    nc.sync.dma_start(
                        lgflat[m * 128 : (m + 1) * 128, chv * VCH : chv * VCH + cw],
                        lg_sb[:, 0:cw])

        for _p in (wp, ps):
            _p.release()

    nc.compile()
    _cache["nc"] = nc
    return nc


def _prep_core_inputs(inputs, core):
    """Host-side layout prep for one core (numpy: pad/transpose/cast only)."""
    sl = slice(core * BL, (core + 1) * BL)
    emb = np.asarray(inputs["trg_seq_embedded"], np.float32)[sl]      # [8, T, E]
    enc_h = np.asarray(inputs["enc_h"], np.float32)[0][sl]            # [8, EH]
    enc_c = np.asarray(inputs["enc_c"], np.float32)[0][sl]
    enc = np.asarray(inputs["encoder_outputs"], np.float32)[sl]       # [8, S, EH]
    mask = np.asarray(inputs["encoder_mask"])[sl]
    W_ih = np.asarray(inputs["W_ih"], np.float32); W_hh = np.asarray(inputs["W_hh"], np.float32)
    b_ih = np.asarray(inputs["b_ih"], np.float32); b_hh = np.asarray(inputs["b_hh"], np.float32)
    W_trans = np.asarray(inputs["W_trans"], np.float32); b_trans = np.asarray(inputs["b_trans"], np.float32)
    W_pre = np.asarray(inputs["W_pre"], np.float32); b_pre = np.asarray(inputs["b_pre"], np.float32)
    W_q = np.asarray(inputs["W_q"], np.float32)
    w_v = np.asarray(inputs["w_v"], np.float32); w_cov = np.asarray(inputs["w_cov"], np.float32)
    W_read = np.asarray(inputs["W_read"], np.float32); b_read = np.asarray(inputs["b_read"], np.float32)
    W_logit = np.asarray(inputs["W_logit"], np.float32); b_logit = np.asarray(inputs["b_logit"], np.float32)

    gscale = np.ones((G4,), np.float32)
    gscale[0:DH] = 0.5; gscale[DH:2 * DH] = 0.5; gscale[3 * DH:] = 0.5

    # embT [128, T*24]
    tmp = np.zeros((T, 384, 8), np.float32)
    tmp[:, 0:E, :] = emb.transpose(1, 2, 0)
    tmp[:, E, :] = 1.0
    embT = tmp.reshape(T, 3, 128, 8).transpose(2, 0, 1, 3).reshape(128, T * 24)

    # WgT [128, 11*2048]
    Wg = np.zeros((KPAD, G4), np.float32)
    Wg[0:E] = W_ih[:, 0:E].T
    Wg[E] = b_ih + b_hh
    Wg[384:896] = W_ih[:, E:E + EH].T
    Wg[896:1408] = W_hh.T
    Wg *= gscale[None, :]
    WgT = Wg.reshape(NKT, 128, G4).transpose(1, 0, 2).reshape(128, NKT * G4)

    # WrT [128, 11*512]  (readout concat order in ref: [emb, h, ctx]);
    # rows permuted so maxout pairs (2k, 2k+1) become halves (k, k+256)
    perm = np.concatenate([np.arange(0, DH, 2), np.arange(1, DH, 2)])
    W_read_p = W_read[perm]
    b_read_p = b_read[perm]
    Wr = np.zeros((KPAD, DH), np.float32)
    Wr[0:E] = W_read_p[:, 0:E].T
    Wr[E] = b_read_p
    Wr[384:896] = W_read_p[:, E + DH:].T        # ctx section
    Wr[896:1408] = W_read_p[:, E:E + DH].T      # h section
    WrT = Wr.reshape(NKT, 128, DH).transpose(1, 0, 2).reshape(128, NKT * DH)

    WqT = W_q.T.reshape(4, 128, A).transpose(1, 0, 2).reshape(128, 4 * A)

    Wtr = np.zeros((5 * 128, DH), np.float32)
    Wtr[0:EH] = W_trans.T
    Wtr[EH] = b_trans
    WtrT = Wtr.reshape(5, 128, DH).transpose(1, 0, 2).reshape(128, 5 * DH)

    ehc = np.zeros((5 * 128, 16), np.float32)
    ehc[0:EH, 0:8] = enc_h.T
    ehc[0:EH, 8:16] = enc_c.T
    ehc[EH, :] = 1.0
    ehcT = ehc.reshape(5, 128, 16).transpose(1, 0, 2).reshape(128, 5 * 16)

    WpT = W_pre.T.reshape(4, 128, 4, 128).transpose(1, 0, 2, 3).reshape(128, 16 * 128)

    encT = enc.transpose(2, 0, 1).reshape(4, 128, BL * S)
    encT = encT.transpose(1, 0, 2).reshape(128, 4 * BL * S)

    encSa = np.zeros((BL, 4, 128, EH), np.float32)
    for ch in range(4):
        cw = min(128, S - ch * 128)
        encSa[:, ch, 0:cw, :] = enc[:, ch * 128 : ch * 128 + cw, :]
    encS = encSa.transpose(2, 0, 1, 3).reshape(128, BL * 4 * EH)

    wvbd = np.zeros((128, 32, 8), np.float32)
    for m in range(4):
        for b in range(BL):
            wvbd[:, m * 8 + b, b] = w_v[m * 128:(m + 1) * 128]
    wvbd = wvbd.reshape(128, 256)

    maskadd = np.where(mask == 1, np.float32(-1e12), np.float32(0.0)).astype(np.float32)
    bpre = b_pre.reshape(4, 128).T.copy()

    WlT = W_logit.T.reshape(2, 128, V).transpose(1, 0, 2).reshape(128, 2 * V)

    cast = lambda x: np.ascontiguousarray(x, np.float32).astype(np.float16)
    return {
        "embT": cast(embT), "WgT": cast(WgT), "WrT": cast(WrT), "WqT": cast(WqT),
        "WtrT": cast(WtrT), "ehcT": cast(ehcT), "WpT": cast(WpT),
        "encT": cast(encT), "encS": cast(encS), "wvbd": cast(wvbd),
        "wcovrow": cast(w_cov.reshape(1, A)), "maskadd": maskadd,
        "bpre": np.ascontiguousarray(bpre, np.float32),
        "WlT": cast(WlT), "blg": cast(b_logit.reshape(1, V)),
    }


def _assemble(results):
    logits = np.empty((T, B, V), np.float32)
    attns = np.empty((T, B, S), np.float32)
    covs = np.empty((T, B, S), np.float32)
    energ = np.empty((T, B, S), np.float32)
    for core, r in enumerate(results):
        sl = slice(core * BL, (core + 1) * BL)
        logits[:, sl, :] = r["logits"]
        attns[:, sl, :] = r["attns"]
        covs[:, sl, :] = r["covs"]
        energ[:, sl, :] = r["energies"]
    return logits, attns, covs, energ


def kernel(**inputs):
    nc = _build_program()
    in_maps = [_prep_core_inputs(inputs, c) for c in range(NC)]
    from concourse.bass_utils import run_bass_kernel_spmd
    trace = bool(int(os.environ.get("KERNEL_TRACE", "0")))
    res = run_bass_kernel_spmd(nc, in_maps, core_ids=list(range(NC)), trace=trace)
    if trace:
        _cache["last_results"] = res
    return _assemble(res.results)


def run_coresim(inputs, core=0):
    """Dev helper: run one core in CoreSim, return that core's outputs dict."""
    nc = _build_program()
    im = _prep_core_inputs(inputs, core)
    from concourse.bass_interp import CoreSim
    sim = CoreSim(nc, trace=False, require_finite=False, require_nnan=False)
    for k, v in im.items():
        sim.tensor(k)[:] = v
    sim.simulate(check_with_hw=False)
    return {k: np.array(sim.tensor(k)) for k in ["logits", "attns", "covs", "energies"]}


# revision 10
# speedup vs baseline: 3.4483x; 3.4483x over previous
"""Trainium2 Bass kernel for nn_DecoderV2 (LSTM decoder + coverage attention +
50k-vocab logit head), SPMD over 8 NeuronCores, batch-sharded.

Self-contained: host-side numpy does layout/padding/casts only; all FLOPs run
on-device. Returns (logits, attns, coverages, energies) like the reference.
"""
import os
import numpy as np

import concourse.bass as bass
import concourse.mybir as mybir
import concourse.tile as tile
from concourse import bacc
from concourse.masks import make_identity

f32 = mybir.dt.float32
f16 = mybir.dt.float16
AF = mybir.ActivationFunctionType
OP = mybir.AluOpType

B, T, S = 64, 32, 400
E, EH, DH, A, V, P = 300, 512, 512, 512, 50000, 2
NC = 8
BL = B // NC          # 8 batch rows per core
KPAD = 1408           # [emb 0:300 | ones@300 | pad | ctx 384:896 | h 896:1408]
NKT = KPAD // 128     # 11 k-tiles
G4 = 4 * DH           # 2048
VCH = 512             # phase-2 vocab chunk
NVCH = (V + VCH - 1) // VCH   # 98

_cache = {}


def _build_program():
    if "nc" in _cache:
        return _cache["nc"]
    nc = bacc.Bacc("TRN2", target_bir_lowering=False, debug=False)

    # ---------------- DRAM I/O ----------------
    d_embT = nc.dram_tensor("embT", [128, T * 24], f16, kind="ExternalInput")
    d_wgT = nc.dram_tensor("WgT", [128, NKT * G4], f16, kind="ExternalInput")
    d_wrT = nc.dram_tensor("WrT", [128, NKT * DH], f16, kind="ExternalInput")
    d_wqT = nc.dram_tensor("WqT", [128, 4 * A], f16, kind="ExternalInput")
    d_wtrT = nc.dram_tensor("WtrT", [128, 5 * DH], f16, kind="ExternalInput")
    d_ehcT = nc.dram_tensor("ehcT", [128, 5 * 16], f16, kind="ExternalInput")
    d_wpT = nc.dram_tensor("WpT", [128, 16 * 128], f16, kind="ExternalInput")
    d_encT = nc.dram_tensor("encT", [128, 4 * BL * S], f16, kind="ExternalInput")
    d_encS = nc.dram_tensor("encS", [128, BL * 4 * EH], f16, kind="ExternalInput")
    d_wvbd = nc.dram_tensor("wvbd", [128, 32 * 8], f16, kind="ExternalInput")
    d_wcov = nc.dram_tensor("wcovrow", [1, A], f16, kind="ExternalInput")
    d_mask = nc.dram_tensor("maskadd", [BL, S], f32, kind="ExternalInput")
    d_bpre = nc.dram_tensor("bpre", [128, 4], f32, kind="ExternalInput")
    d_wlT = nc.dram_tensor("WlT", [128, 2 * V], f16, kind="ExternalInput")
    d_blg = nc.dram_tensor("blg", [1, V], f16, kind="ExternalInput")

    d_logits = nc.dram_tensor("logits", [T, BL, V], f32, kind="ExternalOutput")
    d_attns = nc.dram_tensor("attns", [T, BL, S], f32, kind="ExternalOutput")
    d_covs = nc.dram_tensor("covs", [T, BL, S], f32, kind="ExternalOutput")
    d_energ = nc.dram_tensor("energies", [T, BL, S], f32, kind="ExternalOutput")

    with tile.TileContext(nc) as tc:
        wp = tc.alloc_tile_pool(name="wp", bufs=1)
        lp = tc.alloc_tile_pool(name="lp", bufs=1)

        embT = lp.tile([128, T * 24], f16); nc.sync.dma_start(embT[:], d_embT.ap())
        wgT = lp.tile([128, NKT * G4], f16); nc.sync.dma_start(wgT[:], d_wgT.ap())
        wrT = lp.tile([128, NKT * DH], f16); nc.sync.dma_start(wrT[:], d_wrT.ap())
        wqT = lp.tile([128, 4 * A], f16); nc.sync.dma_start(wqT[:], d_wqT.ap())
        encS = lp.tile([128, BL * 4 * EH], f16); nc.sync.dma_start(encS[:], d_encS.ap())
        wvbd = lp.tile([128, 32 * 8], f16); nc.sync.dma_start(wvbd[:], d_wvbd.ap())
        wcovrow = lp.tile([1, A], f16); nc.sync.dma_start(wcovrow[:], d_wcov.ap())
        maskadd = lp.tile([BL, S], f32); nc.sync.dma_start(maskadd[:], d_mask.ap())
        bpre = lp.tile([128, 4], f32); nc.sync.dma_start(bpre[:], d_bpre.ap())
        ident = wp.tile([128, 128], f16)
        make_identity(nc, ident[:])
        ones128 = wp.tile([1, 128], f16)
        nc.gpsimd.memset(ones128[:], 1.0)

        pre = lp.tile([128, 4 * BL * S], f16)

        # persistent state
        hT = wp.tile([128, 32], f16)
        ctxT = wp.tile([128, 32], f16)
        qT16 = wp.tile([128, 32], f16)
        attnT_bd = wp.tile([128, 4 * 72], f16)
        maxoT = wp.tile([128, 2 * 256], f16)
        c8 = wp.tile([BL, DH], f32)
        cov = wp.tile([BL, S], f32)
        covrow = wp.tile([1, BL * S], f16)

        nc.gpsimd.memset(ctxT[:], 0.0)
        nc.gpsimd.memset(attnT_bd[:], 0.0)
        nc.gpsimd.memset(cov[:], 0.0)
        nc.gpsimd.memset(covrow[:], 0.0)

        # ---------------- init: h0/c0 + pre-compute ----------------
        ps = tc.alloc_tile_pool(name="ps", bufs=2, space="PSUM")
        with tc.tile_pool(name="initsb", bufs=1) as isb:
            wtrT = isb.tile([128, 5 * DH], f16); nc.sync.dma_start(wtrT[:], d_wtrT.ap())
            ehcT = isb.tile([128, 5 * 16], f16); nc.sync.dma_start(ehcT[:], d_ehcT.ap())
            wpT = isb.tile([128, 16 * 128], f16); nc.sync.dma_start(wpT[:], d_wpT.ap())
            encT = isb.tile([128, 4 * BL * S], f16); nc.sync.dma_start(encT[:], d_encT.ap())

            # h0T tiles [128, 8] per m:  sum_k WtrT(k,m).T @ ehcT(k, h-cols)
            for m in range(4):
                hp = ps.tile([128, 8], f32, tag="t")
                for k in range(5):
                    kp = 128 if k < 4 else 1
                    nc.tensor.matmul(
                        hp[:], wtrT[0:kp, k * DH + m * 128 : k * DH + m * 128 + 128],
                        ehcT[0:kp, k * 16 : k * 16 + 8],
                        start=(k == 0), stop=(k == 4))
                nc.vector.tensor_copy(hT[:, m * 8 : m * 8 + 8], hp[:])
            # c0 [8, 512]: sum_k ehcT(k, c-cols).T @ WtrT(k-rowtile)
            cp = ps.tile([BL, DH], f32, tag="a")
            for k in range(5):
                kp = 128 if k < 4 else 1
                nc.tensor.matmul(
                    cp[:], ehcT[0:kp, k * 16 + 8 : k * 16 + 16],
                    wtrT[0:kp, k * DH : (k + 1) * DH],
                    start=(k == 0), stop=(k == 4))
            nc.vector.tensor_copy(c8[:], cp[:])

            # pre[a, (b, s)] = W_pre @ enc.T   (k-tiles over EH)
            for m in range(4):
                for b in range(BL):
                    pp = ps.tile([128, S], f32, tag="a")
                    for k in range(4):
                        nc.tensor.matmul(
                            pp[:], wpT[:, (k * 4 + m) * 128 : (k * 4 + m) * 128 + 128],
                            encT[:, k * (BL * S) + b * S : k * (BL * S) + (b + 1) * S],
                            start=(k == 0), stop=(k == 3))
                    nc.scalar.copy(pre[:, m * (BL * S) + b * S : m * (BL * S) + (b + 1) * S], pp[:])

        # ---------------- the T-step recurrence ----------------
        sp = tc.alloc_tile_pool(name="sp", bufs=1)

        def lhsT_for_k(k, t):
            if k < 3:
                return embT[:, t * 24 + k * 8 : t * 24 + k * 8 + 8]
            if k < 7:
                return ctxT[:, (k - 3) * 8 : (k - 3) * 8 + 8]
            return hT[:, (k - 7) * 8 : (k - 7) * 8 + 8]

        for t in range(T):
            # cov -> f16 row (for wcov rank-1); cov==0 at t==0
            cov16 = sp.tile([BL, S], f16, tag="cov16")
            nc.vector.tensor_copy(cov16[:], cov[:])
            nc.sync.dma_start(
                covrow[0:1, :].rearrange("o (b s) -> o b s", b=BL), cov16[:])

            # gates [8, 2048] computed per 512-chunk (i, f, g, o), tanh each
            thg4 = []
            for n in range(4):
                g_ps = ps.tile([BL, 512], f32, tag="a")
                for k in range(NKT):
                    nc.tensor.matmul(
                        g_ps[:], lhsT_for_k(k, t),
                        wgT[:, k * G4 + n * 512 : k * G4 + (n + 1) * 512],
                        start=(k == 0), stop=(k == NKT - 1))
                thn = sp.tile([BL, 512], f16, tag=f"th{n}")
                nc.scalar.activation(thn[:], g_ps[:], AF.Tanh)
                thg4.append(thn)
            thi, thf, thg, tho = thg4

            # LSTM cell (sigmoid via 0.5*tanh(x/2)+0.5; i/f/o pre-scaled by 0.5 in WgT)
            u = sp.tile([BL, DH], f32, tag="u")
            nc.vector.tensor_scalar(u[:], thf[:], 0.5, 0.5, OP.mult, OP.add)
            c1 = sp.tile([BL, DH], f32, tag="c1")
            nc.vector.tensor_mul(c1[:], u[:], c8[:])
            v = sp.tile([BL, DH], f32, tag="v")
            nc.vector.tensor_scalar(v[:], thi[:], 0.5, 0.5, OP.mult, OP.add)
            w = sp.tile([BL, DH], f32, tag="w")
            nc.vector.tensor_mul(w[:], v[:], thg[:])
            nc.vector.tensor_add(c8[:], c1[:], w[:])
            tanc = sp.tile([BL, DH], f16, tag="tanc")
            nc.scalar.activation(tanc[:], c8[:], AF.Tanh)
            so = sp.tile([BL, DH], f32, tag="so")
            nc.vector.tensor_scalar(so[:], tho[:], 0.5, 0.5, OP.mult, OP.add)
            h16 = sp.tile([BL, DH], f16, tag="h16")
            nc.vector.tensor_mul(h16[:], so[:], tanc[:])

            # hT <- h16.T
            for ch in range(4):
                tp = ps.tile([128, 8], f16, tag="t")
                nc.tensor.transpose(tp[:], h16[:, ch * 128 : (ch + 1) * 128], ident[0:8, 0:8])
                nc.vector.tensor_copy(hT[:, ch * 8 : ch * 8 + 8], tp[:])

            # q = h @ Wq.T -> [8, 512] -> qT16 [128a, 8b] (+ b_pre)
            q_ps = ps.tile([BL, A], f32, tag="a")
            for k in range(4):
                nc.tensor.matmul(q_ps[:], hT[:, k * 8 : k * 8 + 8],
                                 wqT[:, k * A : (k + 1) * A],
                                 start=(k == 0), stop=(k == 3))
            q16 = sp.tile([BL, A], f16, tag="q16")
            nc.vector.tensor_copy(q16[:], q_ps[:])
            for m in range(4):
                tq = ps.tile([128, 8], f16, tag="t")
                nc.tensor.transpose(tq[:], q16[:, m * 128 : (m + 1) * 128], ident[0:8, 0:8])
                nc.vector.tensor_scalar(qT16[:, m * 8 : m * 8 + 8], tq[:],
                                        bpre[:, m : m + 1], None, OP.add)

            # feat = tanh(pre + wcov x cov + q x 1)  [a-part, (b, s)], per-m tile;
            # energy accumulates over m into one [8, 400] psum
            en_ps = ps.tile([BL, S], f32, tag="a")
            for m in range(4):
                featm = sp.tile([128, BL * S], f16, tag="featm")
                for bq in (0, 2, 4, 6):
                    ap = ps.tile([128, 2 * 512], f32, tag="arg")
                    for j in range(2):
                        b = bq + j
                        dst = ap[:, j * 512 : j * 512 + S]
                        nc.tensor.matmul(dst, ident[:],
                                         pre[:, m * (BL * S) + b * S : m * (BL * S) + (b + 1) * S],
                                         start=True, stop=False)
                        nc.tensor.matmul(dst, ident[:],
                                         qT16[:, m * 8 + b : m * 8 + b + 1].broadcast_to((128, S)),
                                         start=False, stop=False)
                    for j in range(2):
                        b = bq + j
                        dst = ap[:, j * 512 : j * 512 + S]
                        nc.tensor.matmul(dst, wcovrow[0:1, m * 128 : (m + 1) * 128],
                                         covrow[0:1, b * S : (b + 1) * S],
                                         start=False, stop=True)
                    asrc = ap[:].rearrange("p (j n) -> p j n", n=512)[:, :, 0:S]
                    dstf = featm[:, bq * S : (bq + 2) * S]
                    nc.scalar.activation(
                        dstf.rearrange("p (j n) -> p j n", n=S), asrc, AF.Tanh)
                for b in range(BL):
                    idx = m * 8 + b
                    nc.tensor.matmul(
                        en_ps[:], wvbd[:, idx * 8 : idx * 8 + 8],
                        featm[:, b * S : (b + 1) * S],
                        start=(idx == 0), stop=(idx == 31))

            # masked energies -> softmax (no max-subtraction; exp(-1e12)=0)
            e8 = sp.tile([BL, S], f32, tag="e8")
            nc.vector.tensor_add(e8[:], en_ps[:], maskadd[:])
            nc.sync.dma_start(d_energ.ap()[t], e8[:])
            exps = sp.tile([BL, S], f32, tag="exps")
            zsum = sp.tile([BL, 1], f32, tag="zsum")
            nc.scalar.activation(exps[:], e8[:], AF.Exp, accum_out=zsum[:])
            zrec = sp.tile([BL, 1], f32, tag="zrec")
            nc.vector.reciprocal(zrec[:], zsum[:])
            attn = sp.tile([BL, S], f32, tag="attn")
            nc.vector.tensor_scalar(attn[:], exps[:], zrec[:], None, OP.mult)
            nc.sync.dma_start(d_attns.ap()[t], attn[:])

            # coverage output (pre-update) then cov += attn
            nc.sync.dma_start(d_covs.ap()[t], cov[:])
            nc.vector.tensor_add(cov[:], cov[:], attn[:])

            # attn -> f16 -> block-diagonal attnT (col b of slot (ch, b))
            attn16 = sp.tile([BL, S], f16, tag="attn16")
            nc.vector.tensor_copy(attn16[:], attn[:])
            for ch in range(4):
                cw = 128 if ch < 3 else S - 384
                ta = ps.tile([128, 8], f16, tag="t")
                nc.tensor.transpose(ta[0:cw, :], attn16[:, ch * 128 : ch * 128 + cw],
                                    ident[0:8, 0:8])
                dst9 = attnT_bd[0:cw, ch * 72 : ch * 72 + 72].rearrange(
                    "p (b n) -> p b n", n=9)[:, :, 0:1]
                nc.vector.tensor_copy(dst9, ta[0:cw, :].rearrange("p (b o) -> p b o", o=1))

            # context: block-diag attn lhsT -> one [8, EH] psum
            ctx_ps = ps.tile([BL, EH], f32, tag="a")
            cidx = 0
            for b in range(BL):
                for ch in range(4):
                    cw = 128 if ch < 3 else S - 384
                    nc.tensor.matmul(
                        ctx_ps[:],
                        attnT_bd[0:cw, ch * 72 + 8 * b : ch * 72 + 8 * b + 8],
                        encS[0:cw, b * (4 * EH) + ch * EH : b * (4 * EH) + (ch + 1) * EH],
                        start=(cidx == 0), stop=(cidx == 31))
                    cidx += 1
            ctx8 = sp.tile([BL, EH], f16, tag="ctx8")
            nc.vector.tensor_copy(ctx8[:], ctx_ps[:])
            for ch in range(4):
                tcx = ps.tile([128, 8], f16, tag="t")
                nc.tensor.transpose(tcx[:], ctx8[:, ch * 128 : (ch + 1) * 128], ident[0:8, 0:8])
                nc.vector.tensor_copy(ctxT[:, ch * 8 : ch * 8 + 8], tcx[:])

            # readout [8, 512] -> maxout -> maxoT
            r_ps = ps.tile([BL, DH], f32, tag="a")
            for k in range(NKT):
                nc.tensor.matmul(r_ps[:], lhsT_for_k(k, t),
                                 wrT[:, k * DH : (k + 1) * DH],
                                 start=(k == 0), stop=(k == NKT - 1))
            r_sb = sp.tile([BL, DH], f32, tag="rsb")
            nc.vector.tensor_copy(r_sb[:], r_ps[:])
            maxo16 = sp.tile([BL, DH // 2], f16, tag="maxo16")
            nc.vector.tensor_max(maxo16[:], r_sb[:, 0:256], r_sb[:, 256:512])
            for ch in range(2):
                tm = ps.tile([128, 8], f16, tag="t")
                nc.tensor.transpose(tm[:], maxo16[:, ch * 128 : (ch + 1) * 128], ident[0:8, 0:8])
                nc.vector.tensor_copy(maxoT[:, ch * 256 + t * 8 : ch * 256 + t * 8 + 8], tm[:])

        # ---------------- phase 2: logits = maxo @ W_logit.T + b_logit ----------------
        sp.release()
        lp.release()
        lgflat = d_logits.ap().rearrange("t b v -> (t b) v")
        with tc.tile_pool(name="lsb", bufs=3) as lsb:
            for chv in range(NVCH):
                cw = min(VCH, V - chv * VCH)
                w0 = lsb.tile([128, VCH], f16, tag="w0")
                nc.sync.dma_start(w0[:, 0:cw], d_wlT.ap()[:, chv * VCH : chv * VCH + cw])
                w1 = lsb.tile([128, VCH], f16, tag="w1")
                nc.sync.dma_start(w1[:, 0:cw], d_wlT.ap()[:, V + chv * VCH : V + chv * VCH + cw])
                bl = lsb.tile([1, VCH], f16, tag="bl")
                nc.sync.dma_start(bl[0:1, 0:cw], d_blg.ap()[0:1, chv * VCH : chv * VCH + cw])
                for m in range(2):
                    lg_ps = ps.tile([128, VCH], f32, tag="a")
                    nc.tensor.matmul(lg_ps[:, 0:cw], maxoT[:, m * 128 : (m + 1) * 128],
                                     w0[:, 0:cw], start=True, stop=False)
                    nc.tensor.matmul(lg_ps[:, 0:cw], maxoT[:, 256 + m * 128 : 256 + (m + 1) * 128],
                                     w1[:, 0:cw], start=False, stop=False)
                    nc.tensor.matmul(lg_ps[:, 0:cw], ones128[0:1, :], bl[0:1, 0:cw],
                                     start=False, stop=True)
                    lg_sb = lsb.tile([128, VCH], f32, tag="lgsb")
                    if m == 0:
                        nc.vector.tensor_copy(lg_sb[:, 0:cw], lg_ps[:, 0:cw])
                    else:
                        nc.scalar.copy(lg_sb[:, 0:cw], lg_ps[:, 0:cw])
                    nc.sync.dma_start(
                        lgflat[m * 128 : (m + 1) * 128, chv * VCH : chv * VCH + cw],
                        lg_sb[:, 0:cw])

        for _p in (wp, ps):
            _p.release()

    nc.compile()
    _cache["nc"] = nc
    return nc


def _prep_core_inputs(inputs, core):
    """Host-side layout prep for one core (numpy: pad/transpose/cast only)."""
    sl = slice(core * BL, (core + 1) * BL)
    emb = np.asarray(inputs["trg_seq_embedded"], np.float32)[sl]      # [8, T, E]
    enc_h = np.asarray(inputs["enc_h"], np.float32)[0][sl]            # [8, EH]
    enc_c = np.asarray(inputs["enc_c"], np.float32)[0][sl]
    enc = np.asarray(inputs["encoder_outputs"], np.float32)[sl]       # [8, S, EH]
    mask = np.asarray(inputs["encoder_mask"])[sl]
    W_ih = np.asarray(inputs["W_ih"], np.float32); W_hh = np.asarray(inputs["W_hh"], np.float32)
    b_ih = np.asarray(inputs["b_ih"], np.float32); b_hh = np.asarray(inputs["b_hh"], np.float32)
    W_trans = np.asarray(inputs["W_trans"], np.float32); b_trans = np.asarray(inputs["b_trans"], np.float32)
    W_pre = np.asarray(inputs["W_pre"], np.float32); b_pre = np.asarray(inputs["b_pre"], np.float32)
    W_q = np.asarray(inputs["W_q"], np.float32)
    w_v = np.asarray(inputs["w_v"], np.float32); w_cov = np.asarray(inputs["w_cov"], np.float32)
    W_read = np.asarray(inputs["W_read"], np.float32); b_read = np.asarray(inputs["b_read"], np.float32)
    W_logit = np.asarray(inputs["W_logit"], np.float32); b_logit = np.asarray(inputs["b_logit"], np.float32)

    gscale = np.ones((G4,), np.float32)
    gscale[0:DH] = 0.5; gscale[DH:2 * DH] = 0.5; gscale[3 * DH:] = 0.5

    # embT [128, T*24]
    tmp = np.zeros((T, 384, 8), np.float32)
    tmp[:, 0:E, :] = emb.transpose(1, 2, 0)
    tmp[:, E, :] = 1.0
    embT = tmp.reshape(T, 3, 128, 8).transpose(2, 0, 1, 3).reshape(128, T * 24)

    # WgT [128, 11*2048]
    Wg = np.zeros((KPAD, G4), np.float32)
    Wg[0:E] = W_ih[:, 0:E].T
    Wg[E] = b_ih + b_hh
    Wg[384:896] = W_ih[:, E:E + EH].T
    Wg[896:1408] = W_hh.T
    Wg *= gscale[None, :]
    WgT = Wg.reshape(NKT, 128, G4).transpose(1, 0, 2).reshape(128, NKT * G4)

    # WrT [128, 11*512]  (readout concat order in ref: [emb, h, ctx]);
    # rows permuted so maxout pairs (2k, 2k+1) become halves (k, k+256)
    perm = np.concatenate([np.arange(0, DH, 2), np.arange(1, DH, 2)])
    W_read_p = W_read[perm]
    b_read_p = b_read[perm]
    Wr = np.zeros((KPAD, DH), np.float32)
    Wr[0:E] = W_read_p[:, 0:E].T
    Wr[E] = b_read_p
    Wr[384:896] = W_read_p[:, E + DH:].T        # ctx section
    Wr[896:1408] = W_read_p[:, E:E + DH].T      # h section
    WrT = Wr.reshape(NKT, 128, DH).transpose(1, 0, 2).reshape(128, NKT * DH)

    WqT = W_q.T.reshape(4, 128, A).transpose(1, 0, 2).reshape(128, 4 * A)

    Wtr = np.zeros((5 * 128, DH), np.float32)
    Wtr[0:EH] = W_trans.T
    Wtr[EH] = b_trans
    WtrT = Wtr.reshape(5, 128, DH).transpose(1, 0, 2).reshape(128, 5 * DH)

    ehc = np.zeros((5 * 128, 16), np.float32)
    ehc[0:EH, 0:8] = enc_h.T
    ehc[0:EH, 8:16] = enc_c.T
    ehc[EH, :] = 1.0
    ehcT = ehc.reshape(5, 128, 16).transpose(1, 0, 2).reshape(128, 5 * 16)

    WpT = W_pre.T.reshape(4, 128, 4, 128).transpose(1, 0, 2, 3).reshape(128, 16 * 128)

    encT = enc.transpose(2, 0, 1).reshape(4, 128, BL * S)
    encT = encT.transpose(1, 0, 2).reshape(128, 4 * BL * S)

    encSa = np.zeros((BL, 4, 128, EH), np.float32)
    for ch in range(4):
        cw = min(128, S - ch * 128)
        encSa[:, ch, 0:cw, :] = enc[:, ch * 128 : ch * 128 + cw, :]
    encS = encSa.transpose(2, 0, 1, 3).reshape(128, BL * 4 * EH)

    wvbd = np.zeros((128, 32, 8), np.float32)
    for m in range(4):
        for b in range(BL):
            wvbd[:, m * 8 + b, b] = w_v[m * 128:(m + 1) * 128]
    wvbd = wvbd.reshape(128, 256)

    maskadd = np.where(mask == 1, np.float32(-1e12), np.float32(0.0)).astype(np.float32)
    bpre = b_pre.reshape(4, 128).T.copy()

    WlT = W_logit.T.reshape(2, 128, V).transpose(1, 0, 2).reshape(128, 2 * V)

    cast = lambda x: np.ascontiguousarray(x, np.float32).astype(np.float16)
    return {
        "embT": cast(embT), "WgT": cast(WgT), "WrT": cast(WrT), "WqT": cast(WqT),
        "WtrT": cast(WtrT), "ehcT": cast(ehcT), "WpT": cast(WpT),
        "encT": cast(encT), "encS": cast(encS), "wvbd": cast(wvbd),
        "wcovrow": cast(w_cov.reshape(1, A)), "maskadd": maskadd,
        "bpre": np.ascontiguousarray(bpre, np.float32),
        "WlT": cast(WlT), "blg": cast(b_logit.reshape(1, V)),
    }


def _assemble(results):
    logits = np.empty((T, B, V), np.float32)
    attns = np.empty((T, B, S), np.float32)
    covs = np.empty((T, B, S), np.float32)
    energ = np.empty((T, B, S), np.float32)
    for core, r in enumerate(results):
        sl = slice(core * BL, (core + 1) * BL)
        logits[:, sl, :] = r["logits"]
        attns[:, sl, :] = r["attns"]
        covs[:, sl, :] = r["covs"]
        energ[:, sl, :] = r["energies"]
    return logits, attns, covs, energ


def kernel(**inputs):
    nc = _build_program()
    in_maps = [_prep_core_inputs(inputs, c) for c in range(NC)]
    from concourse.bass_utils import run_bass_kernel_spmd
    trace = bool(int(os.environ.get("KERNEL_TRACE", "0")))
    res = run_bass_kernel_spmd(nc, in_maps, core_ids=list(range(NC)), trace=trace)
    if trace:
        _cache["last_results"] = res
    return _assemble(res.results)


def run_coresim(inputs, core=0):
    """Dev helper: run one core in CoreSim, return that core's outputs dict."""
    nc = _build_program()
    im = _prep_core_inputs(inputs, core)
    from concourse.bass_interp import CoreSim
    sim = CoreSim(nc, trace=False, require_finite=False, require_nnan=False)
    for k, v in im.items():
        sim.tensor(k)[:] = v
    sim.simulate(check_with_hw=False)
    return {k: np.array(sim.tensor(k)) for k in ["logits", "attns", "covs", "energies"]}
